# Initial kernel scaffold
#
"""DeepSeekMoE (8 experts, top-2, shared expert) on 8 Trainium2 NeuronCores.

Strategy: expert-parallel. Host computes the router (fp32, exact ranking
match with the reference), gathers each expert's tokens, and ships them
(transposed, bf16) to core e = expert e. Each core runs its expert's SwiGLU
over its gathered tokens plus the shared-expert SwiGLU over a 1/8 slice of
all tokens. Matmuls are bf16 with fp32 PSUM accumulation; tokens stream on
the moving (free) dim so weights stay stationary in SBUF for the whole run.
Host applies the fp32 top-2 combine weights and scatters back.
"""

import os
import sys
import math
import numpy as np

sys.path.insert(0, "/opt/trn_rl_repo")

import ml_dtypes

BF16 = ml_dtypes.bfloat16

# Problem constants (hardcoded per harness contract)
T = 16384          # tokens = 4 * 4096
H = 1024           # hidden size
I = 1024           # routed expert intermediate
SI = 1024          # shared expert intermediate
E = 8              # experts
NCORES = 8
TSH = T // NCORES  # shared-expert tokens per core (2048)
SCALE = 1.0
NB = 1024          # token block (DMA granularity)
NT = 512           # matmul free-dim tile (one PSUM bank)
P = 128

_NC_CACHE = {}
LAST_RESULTS = None  # test harness reads exec_time_ns from here


def _build_nc(C):
    import concourse.bacc as bacc
    import concourse.mybir as mybir
    from concourse import tile
    from contextlib import ExitStack

    bf = mybir.dt.bfloat16
    f32 = mybir.dt.float32
    AF = mybir.ActivationFunctionType

    nc = bacc.Bacc("TRN2", target_bir_lowering=False, debug=False,
                   num_devices=NCORES)

    xe = nc.dram_tensor("xe", [H, C], bf, kind="ExternalInput")
    xs = nc.dram_tensor("xs", [H, TSH], bf, kind="ExternalInput")
    wg = nc.dram_tensor("wg", [H, I], bf, kind="ExternalInput")
    wu = nc.dram_tensor("wu", [H, I], bf, kind="ExternalInput")
    wd = nc.dram_tensor("wd", [I, H], bf, kind="ExternalInput")
    sg = nc.dram_tensor("sg", [H, SI], bf, kind="ExternalInput")
    su = nc.dram_tensor("su", [H, SI], bf, kind="ExternalInput")
    sd = nc.dram_tensor("sd", [SI, H], bf, kind="ExternalInput")
    ye = nc.dram_tensor("ye", [H, C], f32, kind="ExternalOutput")
    ys = nc.dram_tensor("ys", [H, TSH], f32, kind="ExternalOutput")

    KO = H // P   # 8 contraction tiles for gate/up
    IO = I // P   # 8 contraction tiles for down / output tiles for gate/up
    HO = H // P   # 8 output tiles for down

    with tile.TileContext(nc) as tc, ExitStack() as ctx:
        wpool = ctx.enter_context(tc.tile_pool(name="w", bufs=1))
        xpool = ctx.enter_context(tc.tile_pool(name="x", bufs=2))
        apool = ctx.enter_context(tc.tile_pool(name="a", bufs=2))
        spool = ctx.enter_context(tc.tile_pool(name="s", bufs=3))
        opool = ctx.enter_context(tc.tile_pool(name="o", bufs=3))
        pgp = ctx.enter_context(tc.tile_pool(name="pg", bufs=2, space="PSUM"))
        pup = ctx.enter_context(tc.tile_pool(name="pu", bufs=2, space="PSUM"))
        pyp = ctx.enter_context(tc.tile_pool(name="py", bufs=2, space="PSUM"))

        wsb = {}
        for name, dram in [("wg", wg), ("wu", wu), ("wd", wd),
                           ("sg", sg), ("su", su), ("sd", sd)]:
            t = wpool.tile([P, KO, dram.shape[1]], bf, tag=name)
            nc.sync.dma_start(t[:], dram.ap().rearrange("(ko p) i -> p ko i", p=P))
            wsb[name] = t

        def ffn(x_dram, y_dram, Tt, gname, uname, dname):
            xv = x_dram.ap().rearrange("(ko p) t -> p ko t", p=P)
            yv = y_dram.ap().rearrange("(ho p) t -> p ho t", p=P)
            for nb in range(Tt // NB):
                xt = xpool.tile([P, KO, NB], bf, tag="xt")
                nc.sync.dma_start(xt[:], xv[:, :, nb * NB:(nb + 1) * NB])
                at = apool.tile([P, IO, NB], bf, tag="at")
                for half in range(NB // NT):
                    hs = slice(half * NT, (half + 1) * NT)
                    for io in range(IO):
                        psg = pgp.tile([P, NT], f32, tag="pg")
                        psu = pup.tile([P, NT], f32, tag="pu")
                        for k in range(KO):
                            nc.tensor.matmul(
                                psg[:], wsb[gname][:, k, io * P:(io + 1) * P],
                                xt[:, k, hs], start=(k == 0), stop=(k == KO - 1))
                        for k in range(KO):
                            nc.tensor.matmul(
                                psu[:], wsb[uname][:, k, io * P:(io + 1) * P],
                                xt[:, k, hs], start=(k == 0), stop=(k == KO - 1))
                        st = spool.tile([P, NT], f32, tag="st")
                        nc.scalar.activation(st[:], psg[:], AF.Silu)
                        nc.vector.tensor_tensor(
                            at[:, io, hs], st[:], psu[:], mybir.AluOpType.mult)
                for ho in range(HO):
                    ot = opool.tile([P, NB], f32, tag="ot")
                    for half in range(NB // NT):
                        hs = slice(half * NT, (half + 1) * NT)
                        psy = pyp.tile([P, NT], f32, tag="py")
                        for io in range(IO):
                            nc.tensor.matmul(
                                psy[:], wsb[dname][:, io, ho * P:(ho + 1) * P],
                                at[:, io, hs], start=(io == 0), stop=(io == IO - 1))
                        nc.vector.tensor_copy(ot[:, hs], psy[:])
                    nc.sync.dma_start(yv[:, ho, nb * NB:(nb + 1) * NB], ot[:])

        ffn(xe, ye, C, "wg", "wu", "wd")
        ffn(xs, ys, TSH, "sg", "su", "sd")

    nc.compile()
    return nc


def _get_nc(C):
    if C not in _NC_CACHE:
        _NC_CACHE[C] = _build_nc(C)
    return _NC_CACHE[C]


def kernel(hidden_states, w_router, w_sh_gate, w_sh_up, w_sh_down,
           w_gate, w_up, w_down):
    global LAST_RESULTS
    from concourse.bass_utils import run_bass_kernel_spmd

    x = np.ascontiguousarray(np.asarray(hidden_states, dtype=np.float32)
                             .reshape(T, H))

    # --- router (fp32, host) ---
    logits = x @ np.asarray(w_router, dtype=np.float32)          # [T, E]
    m = logits.max(axis=1, keepdims=True)
    ex = np.exp(logits - m)
    probs = ex / ex.sum(axis=1, keepdims=True)
    ar = np.arange(T)
    i1 = probs.argmax(axis=1)
    ptmp = probs.copy()
    ptmp[ar, i1] = -1.0
    i2 = ptmp.argmax(axis=1)

    toks, wvs = [], []
    for e in range(E):
        te = np.nonzero((i1 == e) | (i2 == e))[0]
        toks.append(te)
        wvs.append((probs[te, e] * SCALE).astype(np.float32))
    C = max(NB, int(math.ceil(max(len(t) for t in toks) / NB)) * NB)

    # --- build per-core device inputs ---
    x_bf = x.astype(BF16)
    sg_bf = np.asarray(w_sh_gate, dtype=np.float32).astype(BF16)
    su_bf = np.asarray(w_sh_up, dtype=np.float32).astype(BF16)
    sd_bf = np.asarray(w_sh_down, dtype=np.float32).astype(BF16)
    w_gate = np.asarray(w_gate, dtype=np.float32)
    w_up = np.asarray(w_up, dtype=np.float32)
    w_down = np.asarray(w_down, dtype=np.float32)

    in_maps = []
    for e in range(E):
        te = toks[e]
        xg = np.zeros((C, H), BF16)
        xg[:len(te)] = x_bf[te]
        in_maps.append({
            "xe": np.ascontiguousarray(xg.T),
            "xs": np.ascontiguousarray(x_bf[e * TSH:(e + 1) * TSH].T),
            "wg": w_gate[e].astype(BF16),
            "wu": w_up[e].astype(BF16),
            "wd": w_down[e].astype(BF16),
            "sg": sg_bf, "su": su_bf, "sd": sd_bf,
        })

    nc = _get_nc(C)
    res = run_bass_kernel_spmd(nc, in_maps, list(range(NCORES)))
    LAST_RESULTS = res

    # --- host combine ---
    out = np.empty((T, H), np.float32)
    for c in range(NCORES):
        out[c * TSH:(c + 1) * TSH] = res.results[c]["ys"].T
    for e in range(E):
        te = toks[e]
        ye = res.results[e]["ye"][:, :len(te)].T       # [n_e, H]
        out[te] += ye * wvs[e][:, None]

    return out.reshape(hidden_states.shape), logits


# revision 3
# speedup vs baseline: 1.2403x; 1.2403x over previous
"""DeepSeekMoE (8 experts, top-2, shared expert) on 8 Trainium2 NeuronCores.

Strategy: expert-parallel. Host computes the router (fp32, exact ranking
match with the reference), gathers each expert's tokens, and ships them
(transposed, bf16) to core e = expert e. Each core runs its expert's SwiGLU
over its gathered tokens plus the shared-expert SwiGLU over a 1/8 slice of
all tokens. Matmuls are bf16 with fp32 PSUM accumulation; tokens stream on
the moving (free) dim so weights stay stationary in SBUF for the whole run.
Host applies the fp32 top-2 combine weights and scatters back.
"""

import os
import sys
import math
import numpy as np

sys.path.insert(0, "/opt/trn_rl_repo")

# bass_utils' BASS_TRACE path imports antenv.axon_hooks, which some
# containers don't ship. Provide a no-op fallback so tracing degrades
# gracefully instead of crashing the run.
try:
    import antenv.axon_hooks  # noqa: F401
except ImportError:
    import types

    try:
        import antenv
    except ImportError:
        antenv = types.ModuleType("antenv")
        sys.modules["antenv"] = antenv
    _hooks = types.ModuleType("antenv.axon_hooks")
    _hooks.get_axon_ntff_profile_hook = lambda: None
    sys.modules["antenv.axon_hooks"] = _hooks
    antenv.axon_hooks = _hooks

import ml_dtypes

BF16 = ml_dtypes.bfloat16

# Problem constants (hardcoded per harness contract)
T = 16384          # tokens = 4 * 4096
H = 1024           # hidden size
I = 1024           # routed expert intermediate
SI = 1024          # shared expert intermediate
E = 8              # experts
NCORES = 8
TSH = T // NCORES  # shared-expert tokens per core (2048)
SCALE = 1.0
NB = 1024          # token block (DMA granularity)
NT = 512           # matmul free-dim tile (one PSUM bank)
P = 128

_NC_CACHE = {}
LAST_RESULTS = None  # test harness reads exec_time_ns from here


def _build_nc(C):
    import concourse.bacc as bacc
    import concourse.mybir as mybir
    from concourse import tile
    from contextlib import ExitStack

    bf = mybir.dt.bfloat16
    f32 = mybir.dt.float32
    AF = mybir.ActivationFunctionType

    nc = bacc.Bacc("TRN2", target_bir_lowering=False, debug=False,
                   num_devices=NCORES)

    xe = nc.dram_tensor("xe", [H, C], bf, kind="ExternalInput")
    xs = nc.dram_tensor("xs", [H, TSH], bf, kind="ExternalInput")
    wg = nc.dram_tensor("wg", [H, I], bf, kind="ExternalInput")
    wu = nc.dram_tensor("wu", [H, I], bf, kind="ExternalInput")
    wd = nc.dram_tensor("wd", [I, H], bf, kind="ExternalInput")
    sg = nc.dram_tensor("sg", [H, SI], bf, kind="ExternalInput")
    su = nc.dram_tensor("su", [H, SI], bf, kind="ExternalInput")
    sd = nc.dram_tensor("sd", [SI, H], bf, kind="ExternalInput")
    ye = nc.dram_tensor("ye", [H, C], f32, kind="ExternalOutput")
    ys = nc.dram_tensor("ys", [H, TSH], f32, kind="ExternalOutput")

    KO = H // P   # 8 contraction tiles for gate/up
    IO = I // P   # 8 contraction tiles for down / output tiles for gate/up
    HO = H // P   # 8 output tiles for down

    with tile.TileContext(nc) as tc, ExitStack() as ctx:
        wpool = ctx.enter_context(tc.tile_pool(name="w", bufs=1))
        xpool = ctx.enter_context(tc.tile_pool(name="x", bufs=2))
        apool = ctx.enter_context(tc.tile_pool(name="a", bufs=2))
        spool = ctx.enter_context(tc.tile_pool(name="s", bufs=3))
        opool = ctx.enter_context(tc.tile_pool(name="o", bufs=3))
        pgp = ctx.enter_context(tc.tile_pool(name="pg", bufs=2, space="PSUM"))
        pup = ctx.enter_context(tc.tile_pool(name="pu", bufs=2, space="PSUM"))
        pyp = ctx.enter_context(tc.tile_pool(name="py", bufs=2, space="PSUM"))

        wsb = {}
        for name, dram in [("wg", wg), ("wu", wu), ("wd", wd),
                           ("sg", sg), ("su", su), ("sd", sd)]:
            t = wpool.tile([P, KO, dram.shape[1]], bf, tag=name)
            nc.sync.dma_start(t[:], dram.ap().rearrange("(ko p) i -> p ko i", p=P))
            wsb[name] = t

        def ffn(x_dram, y_dram, Tt, gname, uname, dname):
            xv = x_dram.ap().rearrange("(ko p) t -> p ko t", p=P)
            yv = y_dram.ap().rearrange("(ho p) t -> p ho t", p=P)
            for nb in range(Tt // NB):
                xt = xpool.tile([P, KO, NB], bf, tag="xt")
                nc.sync.dma_start(xt[:], xv[:, :, nb * NB:(nb + 1) * NB])
                at = apool.tile([P, IO, NB], bf, tag="at")
                for half in range(NB // NT):
                    hs = slice(half * NT, (half + 1) * NT)
                    for io in range(IO):
                        psg = pgp.tile([P, NT], f32, tag="pg")
                        psu = pup.tile([P, NT], f32, tag="pu")
                        for k in range(KO):
                            nc.tensor.matmul(
                                psg[:], wsb[gname][:, k, io * P:(io + 1) * P],
                                xt[:, k, hs], start=(k == 0), stop=(k == KO - 1))
                        for k in range(KO):
                            nc.tensor.matmul(
                                psu[:], wsb[uname][:, k, io * P:(io + 1) * P],
                                xt[:, k, hs], start=(k == 0), stop=(k == KO - 1))
                        st = spool.tile([P, NT], f32, tag="st")
                        nc.scalar.activation(st[:], psg[:], AF.Silu)
                        nc.vector.tensor_tensor(
                            at[:, io, hs], st[:], psu[:], mybir.AluOpType.mult)
                for ho in range(HO):
                    ot = opool.tile([P, NB], f32, tag="ot")
                    for half in range(NB // NT):
                        hs = slice(half * NT, (half + 1) * NT)
                        psy = pyp.tile([P, NT], f32, tag="py")
                        for io in range(IO):
                            nc.tensor.matmul(
                                psy[:], wsb[dname][:, io, ho * P:(ho + 1) * P],
                                at[:, io, hs], start=(io == 0), stop=(io == IO - 1))
                        nc.vector.tensor_copy(ot[:, hs], psy[:])
                    nc.sync.dma_start(yv[:, ho, nb * NB:(nb + 1) * NB], ot[:])

        ffn(xe, ye, C, "wg", "wu", "wd")
        ffn(xs, ys, TSH, "sg", "su", "sd")

    nc.compile()
    return nc


def _get_nc(C):
    if C not in _NC_CACHE:
        _NC_CACHE[C] = _build_nc(C)
    return _NC_CACHE[C]


def _route_and_pack(hidden_states, w_router, w_sh_gate, w_sh_up, w_sh_down,
                    w_gate, w_up, w_down):
    """Host side: fp32 router + top-2, gather tokens per expert, build
    the per-core device input maps."""
    x = np.ascontiguousarray(np.asarray(hidden_states, dtype=np.float32)
                             .reshape(T, H))

    logits = x @ np.asarray(w_router, dtype=np.float32)          # [T, E]
    m = logits.max(axis=1, keepdims=True)
    ex = np.exp(logits - m)
    probs = ex / ex.sum(axis=1, keepdims=True)
    ar = np.arange(T)
    i1 = probs.argmax(axis=1)
    ptmp = probs.copy()
    ptmp[ar, i1] = -1.0
    i2 = ptmp.argmax(axis=1)

    toks, wvs = [], []
    for e in range(E):
        te = np.nonzero((i1 == e) | (i2 == e))[0]
        toks.append(te)
        wvs.append((probs[te, e] * SCALE).astype(np.float32))
    C = max(NB, int(math.ceil(max(len(t) for t in toks) / NB)) * NB)

    x_bf = x.astype(BF16)
    sg_bf = np.asarray(w_sh_gate, dtype=np.float32).astype(BF16)
    su_bf = np.asarray(w_sh_up, dtype=np.float32).astype(BF16)
    sd_bf = np.asarray(w_sh_down, dtype=np.float32).astype(BF16)
    w_gate = np.asarray(w_gate, dtype=np.float32)
    w_up = np.asarray(w_up, dtype=np.float32)
    w_down = np.asarray(w_down, dtype=np.float32)

    in_maps = []
    for e in range(E):
        te = toks[e]
        xg = np.zeros((C, H), BF16)
        xg[:len(te)] = x_bf[te]
        in_maps.append({
            "xe": np.ascontiguousarray(xg.T),
            "xs": np.ascontiguousarray(x_bf[e * TSH:(e + 1) * TSH].T),
            "wg": w_gate[e].astype(BF16),
            "wu": w_up[e].astype(BF16),
            "wd": w_down[e].astype(BF16),
            "sg": sg_bf, "su": su_bf, "sd": sd_bf,
        })
    return in_maps, C, toks, wvs, logits


def _combine(results, toks, wvs, out_shape):
    out = np.empty((T, H), np.float32)
    for c in range(NCORES):
        out[c * TSH:(c + 1) * TSH] = results[c]["ys"].T
    for e in range(E):
        te = toks[e]
        ye = results[e]["ye"][:, :len(te)].T           # [n_e, H]
        out[te] += ye * wvs[e][:, None]
    return out.reshape(out_shape)


def kernel(hidden_states, w_router, w_sh_gate, w_sh_up, w_sh_down,
           w_gate, w_up, w_down):
    global LAST_RESULTS
    from concourse.bass_utils import run_bass_kernel_spmd

    in_maps, C, toks, wvs, logits = _route_and_pack(
        hidden_states, w_router, w_sh_gate, w_sh_up, w_sh_down,
        w_gate, w_up, w_down)

    nc = _get_nc(C)
    res = run_bass_kernel_spmd(nc, in_maps, list(range(NCORES)))
    LAST_RESULTS = res

    out = _combine(res.results, toks, wvs, hidden_states.shape)
    return out, logits
